# revision 44
# baseline (speedup 1.0000x reference)
"""Trainium2 Bass kernel for DigitConvolutionalModel (conv3x3 -> FC512 -> FC10).

Math: the 3x3 valid conv is linear, so  y_flat = x @ C  with C [784, 676]
holding conv_w values in a banded structure.  Then
    logits = relu(x @ (C @ W1) + b1) @ W2 + b2
The fold W1_eff = C @ W1 is computed on device (banded matmul over only
the nonzero blocks), then the big [2048, 784] @ [784, 512] matmul per
core, relu, and the [*, 512] @ [512, 10] head.  Data-parallel across 8
cores on the batch dim.

v4 schedule: DMA triggers BLOCK the issuing engine's instruction queue
(only 4 outstanding transfers per ring), so all bulk x traffic rides the
compute-free sync + gpsimd rings; scalar carries only 6 early weight
triggers and is free for relus by ~11us; vector carries none.  Weights
go in fold-consumption order as fine-grained tiles (cmb halves + one
tile per W1 m-chunk) so the fold starts ~10us; the fold is
software-pipelined with the first superblock's L1 groups (fold_t+1 runs
between fold_t and L1_t to cover the PSUM->SBUF copy latency); x(0,0..2)
lead on gpsimd so L1 never waits on x; logits leave per superblock so
the final DMA (which gates the fixed ~9us teardown) completes right
after the last L2 piece.
"""

import numpy as np
import ml_dtypes

B = 16384
IMG = 28
K = 3
OUT = IMG - K + 1  # 26
M26 = OUT * OUT  # 676
Q = IMG * IMG  # 784
HID = 512
NCLS = 10

NCORES = 8
BL = B // NCORES  # 2048 rows per core
QT = 112  # q-tile height (partition dim), 7 tiles
NQT = Q // QT  # 7
SB = 512  # batch superblock (matmul N)
NSB = BL // SB  # 4
NHT = HID // 128  # 4
NMC = (M26 + 127) // 128  # 6 m-chunks
NWARM = 7  # dummy matmuls riding out the PE HAM ramp + weight DMA

TRACE = False  # set by test harness to capture an NTFF profile
_CACHE = {}

_BF16 = ml_dtypes.bfloat16


def _band_blocks():
    """Static nonzero block pattern of C^T [676, 784] against (mc, qt) tiling."""
    Cs = np.zeros((Q, M26), dtype=bool)
    ii, jj = np.meshgrid(np.arange(OUT), np.arange(OUT), indexing="ij")
    m = (OUT * ii + jj).ravel()
    for di in range(K):
        for dj in range(K):
            q = ((ii + di) * IMG + (jj + dj)).ravel()
            Cs[q, m] = True
    CT = Cs.T  # [676, 784]
    blocks = []
    for t in range(NQT):
        mcs = []
        for mc in range(NMC):
            rows = min(128, M26 - 128 * mc)
            if CT[128 * mc : 128 * mc + rows, QT * t : QT * (t + 1)].any():
                mcs.append(mc)
        blocks.append(mcs)
    return blocks


_BLOCKS = _band_blocks()
_PAIRS = [(t, mc) for t in range(NQT) for mc in _BLOCKS[t]]
NP_ = len(_PAIRS)  # 14

# cmb pairs split between the two HW rings in fold order
_CM_SPLIT = [(0, 7), (7, NP_)]  # sync: pairs 0-6 (t0..t3), scalar: 7-13


def _build():
    import concourse.bacc as bacc
    import concourse.mybir as mybir
    import concourse.tile as tile

    f32 = mybir.dt.float32
    bf16 = mybir.dt.bfloat16
    AF = mybir.ActivationFunctionType

    nc = bacc.Bacc("TRN2", target_bir_lowering=False, debug=False)

    xt_d = nc.dram_tensor("xt", [Q, BL], bf16, kind="ExternalInput")
    # One combined weight payload per HW ring (cmb pairs + 3 W1 m-chunks,
    # column-concatenated).  A queue runs >2 queued transfers INTERLEAVED
    # across the shared DMA engines, collapsing per-transfer rate, so each
    # ring gets exactly one fat weight transfer (4.6KB rows, ~190GB/s).
    WCOLS = 7 * QT + 3 * HID  # 784 + 1536
    w_d = [
        nc.dram_tensor(f"wring{r}", [128, WCOLS], bf16, kind="ExternalInput")
        for r in range(2)
    ]
    b1_d = nc.dram_tensor("b1l", [128, NHT], f32, kind="ExternalInput")
    w2_d = nc.dram_tensor("w2l", [128, NHT * NCLS], bf16, kind="ExternalInput")
    b2_d = nc.dram_tensor("b2l", [NCLS, 1], f32, kind="ExternalInput")
    out_d = nc.dram_tensor("out", [NCLS, BL], f32, kind="ExternalOutput")

    pair_loc = {}
    for r, (lo, hi) in enumerate(_CM_SPLIT):
        for p in range(lo, hi):
            pair_loc[p] = (r, p - lo)

    with tile.TileContext(nc) as tc:
        with (
            tc.tile_pool(name="weights", bufs=1) as wp,
            tc.tile_pool(name="xin", bufs=1) as xp,
            tc.tile_pool(name="hid", bufs=1) as hp,
            tc.tile_pool(name="lgts", bufs=1) as lp,
            tc.tile_pool(name="psF", bufs=2, space="PSUM") as psF,
            tc.tile_pool(name="ps1", bufs=1, space="PSUM") as ps1p,
            tc.tile_pool(name="ps2", bufs=1, space="PSUM") as ps2p,
        ):
            # ---- PE warmup: scratch memset on gpsimd (free earliest after
            # the entry barrier) so the first matmul fires ~7.1us and the
            # ~5us half-rate PE power ramp finishes as early as possible.
            scratch = wp.tile([128, HID], bf16, tag="scratch")
            nc.vector.memset(scratch[:], 0.0)
            warm = psF.tile([128, HID], f32, tag="ps")
            for i in range(NWARM):
                nc.tensor.matmul(
                    warm[:],
                    lhsT=scratch[:, :128],
                    rhs=scratch[:],
                    start=True,
                    stop=True,
                )

            # ---- DMA plan.  sync: b1 + its weight share + the bulk of x.
            # gpsimd (SWDGE): the first three x(0,*) tiles (needed earliest
            # by the interleaved fold/L1 stream) + the late wide x tiles.
            # scalar: 6 early weight triggers only, free for relus by ~11us.
            xsm, xw = {}, [None] * NQT

            def xload(s, t, eng):
                xx = xp.tile([QT, SB], bf16, tag=f"x{s}_{t}")
                eng.dma_start(
                    out=xx[:],
                    in_=xt_d[QT * t : QT * (t + 1), SB * s : SB * (s + 1)],
                )
                xsm[(s, t)] = xx

            def xwload(t, eng):
                xx = xp.tile([QT, 2 * SB], bf16, tag=f"xw_{t}")
                eng.dma_start(
                    out=xx[:], in_=xt_d[QT * t : QT * (t + 1), 2 * SB : BL]
                )
                xw[t] = xx

            # one combined weight transfer per HW ring; tiny constants last
            # on scalar
            wring = []
            for r, eng in enumerate([nc.sync, nc.scalar]):
                t_ = wp.tile([128, WCOLS], bf16, tag=f"wring{r}", name=f"wring{r}")
                eng.dma_start(out=t_[:], in_=w_d[r][:, :])
                wring.append(t_)

            # Everything that shares a ring with a weight payload is gated
            # behind it (dummy-slot WAW via a scalar copy reading the
            # payload): a queue runs its pending transfers interleaved, so
            # ungated followers steal DMA-engine slots from the weights.
            def wgate(pool, shape, dtype, tag, r):
                g = pool.tile(shape, dtype, tag=tag, name=f"gate_{tag}")
                nc.scalar.activation(g[0:1, 0:1], wring[r][0:1, 0:1], AF.Copy)

            wgate(wp, [128, NHT], f32, "b1", 1)
            b1 = wp.tile([128, NHT], f32, tag="b1")
            nc.scalar.dma_start(out=b1[:], in_=b1_d[:, :])
            wgate(wp, [128, NHT * NCLS], bf16, "w2", 1)
            w2 = wp.tile([128, NHT * NCLS], bf16, tag="w2")
            nc.scalar.dma_start(out=w2[:], in_=w2_d[:, :])
            wgate(wp, [NCLS, 1], f32, "b2", 1)
            b2 = wp.tile([NCLS, 1], f32, tag="b2")
            nc.scalar.dma_start(out=b2[:], in_=b2_d[:, :])

            def cmbslice(r, slot, rows):
                return wring[r][:rows, QT * slot : QT * (slot + 1)]

            def w1slice(mc, rows):
                r, k = divmod(mc, 3)
                base = 7 * QT + k * HID
                return wring[r][:rows, base : base + HID]

            # all x on sync (consumption order).  The late wide tiles go to
            # the SWDGE ring, gated behind the weight payloads (a third
            # concurrently-active queue collapses per-queue DMA throughput,
            # so SWDGE must not start while the weights are in flight).
            # First three x tiles gated behind sync's weight payload: only
            # 4 transfers can be queued per ring, so gating these keeps the
            # weight transfer running ALONE (ungated x03+ triggers then wait
            # the queue-sem rotation naturally).
            for t in range(3):
                wgate(xp, [QT, SB], bf16, f"x0_{t}", 0)
                xload(0, t, nc.sync)
            for t in range(3, NQT):
                xload(0, t, nc.sync)
            for t in range(NQT):
                xload(1, t, nc.sync)
            for t in range(3):
                xwload(t, nc.sync)
            for t in range(3, NQT):
                # SWDGE must not run while the weights are in flight (a
                # third active queue collapses per-queue throughput)
                wgate(xp, [QT, 2 * SB], bf16, f"xw_{t}", 0)
                wgate(xp, [QT, 2 * SB], bf16, f"xw_{t}", 1)
                xwload(t, nc.gpsimd)

            def xslice(s, t):
                if s < 2:
                    return xsm[(s, t)][:]
                return xw[t][:, SB * (s - 2) : SB * (s - 1)]

            # ---- fold: W1_eff[q, h] = sum_m C^T[m, q] * W1[m, h] ----
            # Fold blocks spread over six PSUM banks (psF's two + the four
            # ps1 banks, idle until L1) so the PSUM->SBUF copy latency
            # never blocks the next block's matmuls on bank reuse.
            pair_idx = {pair: i for i, pair in enumerate(_PAIRS)}
            w1eff = [None] * NQT
            FOLD_SLOT = ["ps", "ps", "ps1_0", "ps1_1", "ps1_2", "ps1_3", "ps"]

            def fold_block(t):
                slot = FOLD_SLOT[t]
                pool = psF if slot == "ps" else ps1p
                ps = pool.tile(
                    [QT, HID],
                    f32,
                    tag=slot,
                    name=f"foldps_{t}",
                    bufs=2 if slot == "ps1_0" else (2 if slot == "ps" else 1),
                )
                mcs = _BLOCKS[t]
                for j, mc in enumerate(mcs):
                    rows = min(128, M26 - 128 * mc)
                    p = pair_idx[(t, mc)]
                    pr, pslot = pair_loc[p]
                    nc.tensor.matmul(
                        ps[:],
                        lhsT=cmbslice(pr, pslot, rows),
                        rhs=w1slice(mc, rows),
                        start=(j == 0),
                        stop=(j == len(mcs) - 1),
                    )
                # copy-out on vector only: scalar's queue is stuck behind its
                # weight DMA triggers until ~11us
                we = wp.tile([QT, HID], bf16, tag=f"we{t}", name=f"we{t}")
                nc.vector.tensor_copy(we[:], ps[:])
                w1eff[t] = we

            hs_all = {}
            lg = lp.tile([NCLS, BL], f32, tag="lg")

            def l1_group(s, ht, ps1s):
                for t in range(NQT):
                    nc.tensor.matmul(
                        ps1s[ht][:],
                        lhsT=w1eff[t][:, 128 * ht : 128 * (ht + 1)],
                        rhs=xslice(s, t),
                        start=(t == 0),
                        stop=(t == NQT - 1),
                    )

            def relu(s, ht, ps1s):
                h = hp.tile([128, SB], bf16, tag=f"h{s}_{ht}", name=f"h{s}_{ht}")
                if s == NSB - 1 and ht % 2 == 1:
                    # last superblock: alternate relu engines per ht so the
                    # closing relus run in parallel on scalar and vector
                    nc.vector.tensor_scalar(
                        h[:],
                        ps1s[ht][:],
                        b1[:, ht : ht + 1],
                        0.0,
                        mybir.AluOpType.add,
                        mybir.AluOpType.max,
                    )
                else:
                    nc.scalar.activation(
                        h[:],
                        ps1s[ht][:],
                        AF.Relu,
                        bias=b1[:, ht : ht + 1],
                        scale=1.0,
                    )
                hs_all[(s, ht)] = h

            def alloc_ps1():
                # ps1_0 is double-buffered: superblock s+1's ht0 group never
                # stalls on relu(s,0)'s PSUM read at the block boundary
                return [
                    ps1p.tile(
                        [128, SB],
                        f32,
                        tag=f"ps1_{ht}",
                        name=f"ps1_{ht}",
                        bufs=2 if ht == 0 else 1,
                    )
                    for ht in range(NHT)
                ]

            def l1_block(s):
                ps1s = alloc_ps1()
                if s == 0:
                    # t-outer: consume each x tile in DMA-arrival order
                    for t in range(NQT):
                        for ht in range(NHT):
                            nc.tensor.matmul(
                                ps1s[ht][:],
                                lhsT=w1eff[t][:, 128 * ht : 128 * (ht + 1)],
                                rhs=xslice(s, t),
                                start=(t == 0),
                                stop=(t == NQT - 1),
                            )
                    for ht in range(NHT):
                        relu(s, ht, ps1s)
                    return
                for ht in range(NHT):
                    l1_group(s, ht, ps1s)
                    relu(s, ht, ps1s)

            def l2_block(s):
                ps2 = ps2p.tile([NCLS, SB], f32, tag="ps2a", name=f"ps2_{s}")
                for ht in range(NHT):
                    nc.tensor.matmul(
                        ps2[:],
                        lhsT=w2[:, NCLS * ht : NCLS * (ht + 1)],
                        rhs=hs_all[(s, ht)][:],
                        start=(ht == 0),
                        stop=(ht == NHT - 1),
                    )
                half = SB // 2
                lo = SB * s
                nc.vector.tensor_scalar(
                    lg[:, lo : lo + half],
                    ps2[:, :half],
                    b2[:, 0:1],
                    None,
                    mybir.AluOpType.add,
                )
                nc.scalar.activation(
                    lg[:, lo + half : lo + SB],
                    ps2[:, half:],
                    AF.Identity,
                    bias=b2[:, 0:1],
                    scale=1.0,
                )
                eng = nc.sync if s % 2 == 0 else nc.scalar
                eng.dma_start(out=out_d[:, lo : lo + SB], in_=lg[:, lo : lo + SB])

            def l2_last():
                # s=3 in two half-N pieces sharing one PSUM bank: shorter
                # closing chain, halves' bias+DMA on independent engine pairs
                s = NSB - 1
                half = SB // 2
                lo = SB * s
                ps2 = ps2p.tile([NCLS, SB], f32, tag="ps2a", name="ps2_3")
                for ht in range(NHT):
                    nc.tensor.matmul(
                        ps2[:, :half],
                        lhsT=w2[:, NCLS * ht : NCLS * (ht + 1)],
                        rhs=hs_all[(s, ht)][:, :half],
                        start=(ht == 0),
                        stop=(ht == NHT - 1),
                    )
                nc.vector.tensor_scalar(
                    lg[:, lo : lo + half],
                    ps2[:, :half],
                    b2[:, 0:1],
                    None,
                    mybir.AluOpType.add,
                )
                nc.sync.dma_start(
                    out=out_d[:, lo : lo + half], in_=lg[:, lo : lo + half]
                )
                for ht in range(NHT):
                    nc.tensor.matmul(
                        ps2[:, half:],
                        lhsT=w2[:, NCLS * ht : NCLS * (ht + 1)],
                        rhs=hs_all[(s, ht)][:, half:],
                        start=(ht == 0),
                        stop=(ht == NHT - 1),
                    )
                nc.scalar.activation(
                    lg[:, lo + half : lo + SB],
                    ps2[:, half:],
                    AF.Identity,
                    bias=b2[:, 0:1],
                    scale=1.0,
                )
                nc.scalar.dma_start(
                    out=out_d[:, lo + half : lo + SB],
                    in_=lg[:, lo + half : lo + SB],
                )

            # ---- PE stream: fold blocks, then the four L1 superblocks
            # with each L2 deferred one block.
            for t in range(NQT):
                fold_block(t)
            l1_block(0)
            l1_block(1)
            l2_block(0)
            l1_block(2)
            l2_block(1)
            l1_block(3)
            l2_block(2)
            l2_last()

    nc.compile()
    return nc


def _get_nc():
    if "nc" not in _CACHE:
        _CACHE["nc"] = _build()
    return _CACHE["nc"]


def kernel(x, conv_w, W1, b1, W2, b2):
    from concourse.bass_utils import run_bass_kernel_spmd

    nc = _get_nc()

    # C [784, 676]: y_flat = x @ C  (banded placement of conv_w values)
    C = np.zeros((Q, M26), dtype=np.float32)
    ii, jj = np.meshgrid(np.arange(OUT), np.arange(OUT), indexing="ij")
    m = (OUT * ii + jj).ravel()
    cw = np.asarray(conv_w, dtype=np.float32)
    for di in range(K):
        for dj in range(K):
            q = ((ii + di) * IMG + (jj + dj)).ravel()
            C[q, m] = cw[di, dj]
    CT = C.T  # [676, 784]
    w1f = np.asarray(W1, np.float32)
    # combined per-ring weight payloads: 7 cmb pair blocks + 3 W1 m-chunks
    WCOLS = 7 * QT + 3 * HID
    w_pieces = []
    for r, (lo, hi) in enumerate(_CM_SPLIT):
        piece = np.zeros((128, WCOLS), dtype=np.float32)
        for k, p in enumerate(range(lo, hi)):
            t, mc = _PAIRS[p]
            rows = min(128, M26 - 128 * mc)
            piece[:rows, QT * k : QT * (k + 1)] = CT[
                128 * mc : 128 * mc + rows, QT * t : QT * (t + 1)
            ]
        for k, mc in enumerate(range(3 * r, 3 * r + 3)):
            rows = min(128, M26 - 128 * mc)
            base = 7 * QT + k * HID
            piece[:rows, base : base + HID] = w1f[128 * mc : 128 * mc + rows, :]
        w_pieces.append(piece.astype(_BF16))

    b1l = np.ascontiguousarray(
        np.asarray(b1, np.float32).reshape(NHT, 128).T
    )  # [128, 4]
    w2l = np.ascontiguousarray(
        np.asarray(W2, np.float32)
        .reshape(NHT, 128, NCLS)
        .transpose(1, 0, 2)
        .reshape(128, NHT * NCLS)
    ).astype(_BF16)
    b2l = np.asarray(b2, np.float32).reshape(NCLS, 1)

    xf = np.asarray(x, np.float32)
    in_maps = []
    for c in range(NCORES):
        xt = np.ascontiguousarray(xf[c * BL : (c + 1) * BL].T).astype(_BF16)
        im = {"xt": xt, "b1l": b1l, "w2l": w2l, "b2l": b2l}
        for r in range(2):
            im[f"wring{r}"] = w_pieces[r]
        in_maps.append(im)

    kwargs = {}
    if TRACE:
        import profhook  # noqa: F401  (installs the NTFF hook shim)
        import tempfile

        kwargs = {"trace": True, "tmpdir": tempfile.mkdtemp(prefix="ntff_")}
    res = run_bass_kernel_spmd(nc, in_maps, core_ids=list(range(NCORES)), **kwargs)
    if TRACE:
        _CACHE["last_results"] = res

    out = np.concatenate(
        [np.ascontiguousarray(res.results[c]["out"].T) for c in range(NCORES)], axis=0
    ).astype(np.float32)
    return out
